# revision 52
# baseline (speedup 1.0000x reference)
"""GPT2-style fused causal attention (DecisionTransformer) on 8 Trainium2
NeuronCores — v3.

Sharding: tensor-parallel over the 16 heads (2 heads / core, both batches on
every core), row-parallel output projection; host sums the 8 partials.

vs baseline (261 us):
  - X is transposed + cast to bf16 on the host: the kernel streams X^T
    directly from DRAM (8.4 MB instead of 16.8) and the 256 PE transposes
    (~42 us of tensor-engine time) disappear entirely.
  - QKV is weight-stationary bf16: LDWEIGHTS once per (fc, ko) covering 4
    token-chunk PSUM banks (bf16 FWL loads serialize with matmuls, so they
    must be amortized; f32r loads overlap and don't).
  - Attention runs in f32r with K=128 zero-padded scores and 128-wide
    V_aug: sub-full-array matmuls (K=64 / M=66) don't register as activity
    in the PE clock gate and leave the array throttled at 1.2 GHz.
  - The scalar engine runs ONLY the exp (the attention-phase bottleneck);
    batch-1 QKV and the projections interleave into the attention phases.
"""

import sys

for _p in ("/opt/trn_rl_repo",):
    if _p not in sys.path:
        sys.path.insert(0, _p)

import numpy as np
import ml_dtypes

import concourse.bass as bass
import concourse.mybir as mybir
import concourse.tile as tile
from concourse import bacc
from concourse.bass_utils import run_bass_kernel_spmd

P = 128
B, S, D, H, HD = 2, 2048, 1024, 16, 64
T = B * S              # 4096 tokens
KO = D // P            # 8 contraction chunks
QC = 512               # query chunk
NQC = S // QC          # 4
NKB = S // P           # 16 key blocks per sequence
SCALE = 1.0 / float(HD) ** 0.5
N_CORES = 8
HPC = H // N_CORES     # 2 heads per core

f32 = mybir.dt.float32
f32r = mybir.dt.float32r
bf16 = mybir.dt.bfloat16
BF = ml_dtypes.bfloat16


def _build_program():
    nc = bacc.Bacc(None, target_bir_lowering=False)

    xt_d = nc.dram_tensor("xt", [D, T], bf16, kind="ExternalInput")
    wqkv_d = nc.dram_tensor("w_qkv", [P, KO * 3 * P], bf16, kind="ExternalInput")
    bqkv_d = nc.dram_tensor("b_qkv", [P, 3], f32, kind="ExternalInput")
    wp_d = nc.dram_tensor("w_proj", [P, D], bf16, kind="ExternalInput")
    mask_d = nc.dram_tensor("mask", [P, P], f32, kind="ExternalInput")
    id2_d = nc.dram_tensor("id2", [P, HD], bf16, kind="ExternalInput")
    e2_d = nc.dram_tensor("e2", [2, P], f32, kind="ExternalInput")
    out_d = nc.dram_tensor("out", [T, D], bf16, kind="ExternalOutput")

    with tile.TileContext(nc) as tc:
        with (
            tc.tile_pool(name="const", bufs=1) as const,
            tc.tile_pool(name="pt", bufs=6) as pt_pool,
            tc.tile_pool(name="atn", bufs=7) as atn_pool,
            tc.tile_pool(name="den", bufs=2) as den_pool,
            tc.tile_pool(name="rbs", bufs=3) as rbs_pool,
            tc.tile_pool(name="ot", bufs=2) as ot_pool,
            tc.tile_pool(name="ps_a", bufs=2, space="PSUM") as ps_a,
            tc.tile_pool(name="ps_sc", bufs=3, space="PSUM") as ps_sc,
            tc.tile_pool(name="ps_po", bufs=3, space="PSUM") as ps_po,
        ):
            # ---- constants (all host-prepared) ----
            mask_st = const.tile([P, P], f32)
            nc.sync.dma_start(mask_st[:], mask_d[:])
            mask_sb = const.tile([P, P], f32r)
            nc.vector.tensor_copy(mask_sb[:], mask_st[:])
            wqkv_sb = const.tile([P, KO * 3 * P], bf16)
            nc.sync.dma_start(wqkv_sb[:], wqkv_d[:])
            wp_st = const.tile([P, D], bf16)
            nc.sync.dma_start(wp_st[:], wp_d[:])
            wp_sb = const.tile([P, D], f32r)
            nc.vector.tensor_copy(wp_sb[:], wp_st[:])
            bqkv_sb = const.tile([P, 3], f32)
            nc.sync.dma_start(bqkv_sb[:], bqkv_d[:])
            id2_sb = const.tile([P, HD], bf16)
            nc.sync.dma_start(id2_sb[:], id2_d[:])
            e2_st = const.tile([2, P], f32)
            nc.sync.dma_start(e2_st[:], e2_d[:])
            e2_sb = const.tile([2, P], f32r)
            nc.vector.tensor_copy(e2_sb[:], e2_st[:])

            # persistent SBUF state (split per batch / per ko so the tile
            # dependency tracking stays fine-grained)
            xts = [
                [const.tile([P, S], bf16, name=f"xts{b}_{ko}") for ko in range(KO)]
                for b in range(B)
            ]
            # zero-padded Q^T per (batch, head): the other head's 64 rows
            # stay zero so full-128-contraction scores matmuls are exact
            qpad = [
                [const.tile([P, S], f32r, name=f"qp{b}{h}") for h in range(HPC)]
                for b in range(B)
            ]
            kT = [const.tile([P, S], f32r, name=f"kT{b}") for b in range(B)]
            vT = [const.tile([P, S], bf16, name=f"vT{b}") for b in range(B)]
            vaug = [
                const.tile([P, NKB, P], f32r, name=f"vaug{p}")
                for p in range(B * HPC)
            ]
            for b in range(B):
                nc.vector.memset(qpad[b][0][HD:, :].bitcast(f32), 0.0)
                nc.vector.memset(qpad[b][1][:HD, :].bitcast(f32), 0.0)
            for p in range(B * HPC):
                nc.vector.memset(vaug[p][:, :, HD : HD + 1].bitcast(f32), 1.0)
                nc.vector.memset(vaug[p][:, :, HD + 1 :].bitcast(f32), 0.0)

            # HAM warmup: ~8us of matmul activity while DMAs stream in
            for w in range(28):
                psw = ps_sc.tile([P, QC], f32, tag="sc", name="psw")
                nc.tensor.matmul(
                    psw[:, :P], mask_sb[:], mask_sb[:], start=True, stop=True
                )

            # X^T loads: batch 0 in half-chunks (first wave starts after
            # ~1/2 of ko=0 lands), batch 1 as single 4KB-line transfers
            for t2 in range(2):
                for ko in range(KO):
                    nc.sync.dma_start(
                        xts[0][ko][:, t2 * S // 2 : (t2 + 1) * S // 2],
                        xt_d[
                            ko * P : (ko + 1) * P,
                            t2 * S // 2 : (t2 + 1) * S // 2,
                        ],
                    )
            for ko in range(KO):
                nc.sync.dma_start(
                    xts[1][ko][:],
                    xt_d[ko * P : (ko + 1) * P, S : 2 * S],
                )

            # ---------------- emit helpers ----------------
            def qkv_wave(b, fc, half):
                """One weight-stationary wave: 2 PSUM banks (2 token chunks
                of 512), accumulating over all 8 ko; LDW per (fc, ko)."""
                ps = [
                    ps_a.tile([P, QC], f32, tag="a", name=f"qkv{b}{fc}{half}{t}")
                    for t in range(2)
                ]
                for ko in range(KO):
                    wcol = ko * 3 * P + fc * P
                    for t in range(2):
                        c0 = (half * 2 + t) * QC
                        nc.tensor.matmul(
                            ps[t][:],
                            wqkv_sb[:, wcol : wcol + P],
                            xts[b][ko][:, c0 : c0 + QC],
                            start=(ko == 0),
                            stop=(ko == KO - 1),
                        )
                # batch-0 evictions ride the idle scalar engine; batch-1
                # waves run inside batch-0's exp-bound attention window, so
                # their evictions stay on the vector engine
                def evict(dst_ap, src_ap, bias_ap, i=[0]):
                    if b == 0 or i[0] % 2 == 0:
                        nc.scalar.activation(
                            dst_ap, src_ap,
                            mybir.ActivationFunctionType.Identity,
                            bias=bias_ap,
                        )
                    else:
                        nc.vector.tensor_scalar(
                            dst_ap, src_ap, bias_ap, None, mybir.AluOpType.add
                        )
                    i[0] += 1

                for t in range(2):
                    c0 = (half * 2 + t) * QC
                    cs = slice(c0, c0 + QC)
                    if fc == 0:
                        evict(qpad[b][0][:HD, cs], ps[t][:HD], bqkv_sb[:HD, 0:1])
                        evict(qpad[b][1][HD:, cs], ps[t][HD:], bqkv_sb[HD:, 0:1])
                    else:
                        dst = kT[b] if fc == 1 else vT[b]
                        evict(dst[:, cs], ps[t][:], bqkv_sb[:, fc : fc + 1])

            def vaug_build(b):
                """V natural layout (+ ones col) from V^T via PE transposes."""
                for hl in range(HPC):
                    p = b * HPC + hl
                    vt = vT[b][hl * HD : (hl + 1) * HD, :]
                    for kb in range(0, NKB, 2):
                        ps = ps_a.tile(
                            [P, 2, HD], bf16, tag="a", name=f"va{p}{kb}"
                        )
                        for u in range(2):
                            c0 = (kb + u) * P
                            nc.tensor.transpose(
                                ps[:, u, :],
                                vt[:, c0 : c0 + P],
                                id2_sb[hl * HD : (hl + 1) * HD, :],
                            )
                        nc.vector.tensor_copy(
                            vaug[p][:, kb : kb + 2, :HD], ps[:]
                        )

            atn = [[None] * NQC for _ in range(B)]

            def attn_qc(b, qc, filler=None):
                """Causal attention for both heads of batch b, query chunk
                qc: padded K=128 scores, exp on ACT, AV accumulate,
                normalize via ones-broadcast reciprocal."""
                nkb = (qc + 1) * (QC // P)
                po = [
                    ps_po.tile([P, QC], f32, tag="po", name=f"po{b}{qc}{h}")
                    for h in range(HPC)
                ]
                for kb in range(nkb):
                    j = kb - qc * (QC // P)
                    lo = j * P if j > 0 else 0
                    pts = []
                    for hl in range(HPC):
                        sc = ps_sc.tile([P, QC], f32, tag="sc", name=f"sc{hl}")
                        nc.tensor.matmul(
                            sc[:, lo:],
                            kT[b][:, kb * P : (kb + 1) * P],
                            qpad[b][hl][:, qc * QC + lo : (qc + 1) * QC],
                            start=True,
                            stop=True,
                        )
                        pt = pt_pool.tile([P, QC], f32r, tag="pt", name=f"pt{hl}")
                        nc.scalar.activation(
                            pt[:, lo:], sc[:, lo:],
                            mybir.ActivationFunctionType.Exp, scale=SCALE,
                        )
                        if j >= 0:
                            nc.vector.tensor_tensor(
                                pt[:, j * P : (j + 1) * P],
                                pt[:, j * P : (j + 1) * P],
                                mask_sb[:],
                                mybir.AluOpType.mult,
                            )
                        pts.append(pt)
                    for hl in range(HPC):
                        nc.tensor.matmul(
                            po[hl][:, lo:],
                            vaug[b * HPC + hl][:, kb, :],
                            pts[hl][:, lo:],
                            start=(kb == 0),
                            stop=(kb == nkb - 1),
                        )
                    if filler is not None:
                        next(filler, None)
                # normalize: den rows -> broadcast via ones matmul -> recip -> mult
                at = atn_pool.tile([P, QC], f32r, tag="atn", name=f"atn{b}{qc}")
                for hl in range(HPC):
                    hp = slice(hl * HD, (hl + 1) * HD)
                    den = den_pool.tile([1, QC], f32, tag="den", name=f"den{hl}")
                    nc.vector.tensor_copy(den[:], po[hl][HD : HD + 1, :])
                    # broadcast the denominator row across 64 partitions on
                    # the (otherwise idle) gpsimd engine, off the PE
                    bc = rbs_pool.tile([HD, QC], f32, tag="rb", name=f"bc{hl}")
                    nc.gpsimd.partition_broadcast(bc[:], den[:], channels=HD)
                    rb = rbs_pool.tile([HD, QC], f32, tag="rb", name=f"rb{hl}")
                    nc.vector.reciprocal_approx_fast(out=rb[:], in_=bc[:])
                    nc.vector.tensor_tensor(
                        at[hp, :], po[hl][:HD, :], rb[:], mybir.AluOpType.mult
                    )
                atn[b][qc] = at

            def proj_units(jobs):
                """Generator: one (qb, nck-pair) projection unit per next()
                so projections interleave into attention kb iterations."""
                for b_, qc_, eoa in jobs:
                    for qb in range(QC // P):
                        ot = ot_pool.tile([P, D], bf16, tag="ot", name="ot")
                        for nck in range(2):
                            pp = ps_a.tile(
                                [P, D // 2], f32, tag="a",
                                name=f"pp{b_}{qc_}{qb}{nck}",
                            )
                            nc.tensor.matmul(
                                pp[:],
                                atn[b_][qc_][:, qb * P : (qb + 1) * P],
                                wp_sb[:, nck * (D // 2) : (nck + 1) * (D // 2)],
                                start=True,
                                stop=True,
                            )
                            dst = ot[:, nck * (D // 2) : (nck + 1) * (D // 2)]
                            if eoa:
                                nc.scalar.copy(dst, pp[:])
                            else:
                                nc.vector.tensor_copy(dst, pp[:])
                        row = b_ * S + qc_ * QC + qb * P
                        nc.sync.dma_start(out_d[row : row + P, :], ot[:])
                        yield

            def proj_qc(b, qc, evict_on_act=False):
                for qb in range(QC // P):
                    ot = ot_pool.tile([P, D], bf16, tag="ot", name="ot")
                    for nck in range(2):
                        pp = ps_a.tile(
                            [P, D // 2], f32, tag="a", name=f"pp{b}{qc}{qb}{nck}"
                        )
                        nc.tensor.matmul(
                            pp[:],
                            atn[b][qc][:, qb * P : (qb + 1) * P],
                            wp_sb[:, nck * (D // 2) : (nck + 1) * (D // 2)],
                            start=True,
                            stop=True,
                        )
                        dst = ot[:, nck * (D // 2) : (nck + 1) * (D // 2)]
                        if evict_on_act:
                            nc.scalar.copy(dst, pp[:])
                        else:
                            nc.vector.tensor_copy(dst, pp[:])
                    row = b * S + qc * QC + qb * P
                    if b == 1 and qc >= 2:
                        # tail: split across queues so the final flush
                        # doesn't serialize behind one DMA ring
                        nc.sync.dma_start(
                            out_d[row : row + P, : D // 2], ot[:, : D // 2]
                        )
                        nc.sync.dma_start(
                            out_d[row : row + P, D // 2 :], ot[:, D // 2 :]
                        )
                    else:
                        nc.sync.dma_start(out_d[row : row + P, :], ot[:])

            # ---------------- schedule ----------------
            for fc in range(3):
                for half in range(2):
                    qkv_wave(0, fc, half)
            vaug_build(0)
            # b0 attention interleaved with b1 qkv (ACT-bound phase: feed
            # the PE with b1's projection waves between query chunks)
            attn_qc(0, 0)
            qkv_wave(1, 0, 0)
            qkv_wave(1, 0, 1)
            attn_qc(0, 1)
            qkv_wave(1, 1, 0)
            qkv_wave(1, 1, 1)
            attn_qc(0, 2)
            qkv_wave(1, 2, 0)
            qkv_wave(1, 2, 1)
            vaug_build(1)
            attn_qc(0, 3)
            # b1 attention interleaved with projections, staggered so only
            # the final qc's projection remains after the last attention
            attn_qc(1, 0)
            proj_qc(0, 0)
            attn_qc(1, 1)
            proj_qc(0, 1)
            proj_qc(1, 0)
            attn_qc(1, 2)
            proj_qc(0, 2)
            proj_qc(1, 1)
            attn_qc(1, 3)
            proj_qc(0, 3)
            proj_qc(1, 2, evict_on_act=True)
            proj_qc(1, 3, evict_on_act=True)

    nc.compile()
    return nc


_CACHE = {}


def get_program():
    if "nc" not in _CACHE:
        _CACHE["nc"] = _build_program()
    return _CACHE["nc"]


def make_in_maps(hidden_states, c_attn_w, c_attn_b, c_proj_w):
    x = np.asarray(hidden_states, dtype=np.float32).reshape(T, D)
    xt = np.ascontiguousarray(x.T).astype(BF)                     # [D, T]
    wa = np.asarray(c_attn_w, dtype=np.float32)
    ba = np.asarray(c_attn_b, dtype=np.float32)
    wp = np.asarray(c_proj_w, dtype=np.float32)

    kk, qq = np.meshgrid(np.arange(P), np.arange(P), indexing="ij")
    mask = (kk <= qq).astype(np.float32)                          # [P, P]
    r, c = np.meshgrid(np.arange(P), np.arange(HD), indexing="ij")
    id2 = ((r == c) | (r == c + HD)).astype(BF)                   # [P, HD]
    e2 = np.ones((2, P), dtype=np.float32)

    in_maps = []
    for core in range(N_CORES):
        lo = core * P
        # [d, fc, i] -> [p, ko, fc, i] -> [P, KO*3*P]
        wa3 = np.stack(
            [wa[:, lo : lo + P], wa[:, D + lo : D + lo + P],
             wa[:, 2 * D + lo : 2 * D + lo + P]],
            axis=1,
        )                                                          # [D, 3, P]
        wq = np.ascontiguousarray(
            wa3.reshape(KO, P, 3, P).transpose(1, 0, 2, 3).reshape(P, KO * 3 * P)
        ).astype(BF)
        bq = np.ascontiguousarray(
            np.stack(
                [ba[lo : lo + P], ba[D + lo : D + lo + P],
                 ba[2 * D + lo : 2 * D + lo + P]],
                axis=1,
            )
        ).astype(np.float32)                                       # [P, 3]
        wpc = np.ascontiguousarray(wp[lo : lo + P, :]).astype(BF)  # [P, D]
        in_maps.append(
            {
                "xt": xt,
                "w_qkv": wq,
                "b_qkv": bq,
                "w_proj": wpc,
                "mask": mask,
                "id2": id2,
                "e2": e2,
            }
        )
    return in_maps


def kernel(hidden_states, c_attn_w, c_attn_b, c_proj_w, c_proj_b):
    nc = get_program()
    in_maps = make_in_maps(hidden_states, c_attn_w, c_attn_b, c_proj_w)
    res = run_bass_kernel_spmd(nc, in_maps, list(range(N_CORES)))
    acc = res.results[0]["out"].astype(np.float32)
    for core in range(1, N_CORES):
        acc = acc + res.results[core]["out"]
    acc = acc + np.asarray(c_proj_b, dtype=np.float32)[None, :]
    return acc.reshape(B, S, D).astype(np.float32)


if __name__ == "__main__":
    rng = np.random.default_rng(0)
    hs = rng.standard_normal((B, S, D), dtype=np.float32)
    wa = rng.standard_normal((D, 3 * D), dtype=np.float32) * 0.02
    ba = rng.standard_normal((3 * D,), dtype=np.float32) * 0.02
    wp = rng.standard_normal((D, D), dtype=np.float32) * 0.02
    bp = rng.standard_normal((D,), dtype=np.float32) * 0.02
    out = kernel(hs, wa, ba, wp, bp)
    print("out", out.shape, out.dtype, float(np.abs(out).max()))
